# revision 13
# baseline (speedup 1.0000x reference)
"""Bass/Trainium2 kernel for the 2-branch GCN (gnn_message_passing).

Computation (reference):
    per branch i in {a, b}:
        u_i = x_i @ W1_i                                  [N, H]
        h_i = relu(spmm(A, u_i) + b1_i)                   [N, H]
        v_i = h_i @ W2_i                                  [N, H]
        g_i = spmm(A, v_i) + b2_i                         [N, H]
        z_i = log_softmax(g_i @ LW_i + Lb_i)              [N, H]
    out = log_softmax(concat(z_a, z_b) @ LW + Lb)         [N, C]
where spmm(A, u)[d] = sum_{e: dst[e]=d} w[e] * u[src[e]].

Strategy (8 NeuronCores, node-sharded; fp8 message tables):
  - Core c owns node rows [c*S, (c+1)*S), S = N/8.
  - SpMM: AllGather the (concat-branch) activation table [N, 2H] in
    fp8e4m3, then per 128-dst tile: one indirect-DMA row gather per
    source half (int16 indices;
    pad slots gather row 0 with zero weight), then DoubleRow fp8 matmuls
    against a host-precomputed banded aggregation matrix M
    (M[slot, dst_local] = 64*w[e], fp8e4m3), 256 edge slots per chunk,
    accumulating in one [128, 2H] PSUM bank per tile.
  - The x64 edge-weight scale (fp8 denormal avoidance) and a x16 scale
    on the layer-2 table are folded into W2/LW host-side, so the device
    computes exact multiples of the reference quantities.
  - Everything is tile-local: dense phases are interleaved per-tile
    into the spmm loops (PE fills gather-wait bubbles), and h/g/z live
    in small rotating SBUF tiles, not full-length feature-major arrays.
  - Both branches share each gather (concat features -> 512B fp8 rows).
  - M / index tensors depend only on the graph: built once on the host,
    M streamed per tile, indices resident in SBUF.
"""

import sys

if "/opt/trn_rl_repo" not in sys.path:
    sys.path.insert(0, "/opt/trn_rl_repo")

import numpy as np
import ml_dtypes

import concourse.bass as bass
import concourse.bacc as bacc
import concourse.mybir as mybir
import concourse.tile as tile
from concourse.tile import TileContext
from concourse.masks import make_identity
from concourse.bass_utils import run_bass_kernel_spmd

import contextlib
import concourse.bacc as _bacc_mod


@contextlib.contextmanager
def _pinned_act_tables():
    """During compile, make every activation-function table except the
    all-purpose one look empty so bacc's table-load inserter picks a single
    table for the whole program (one LoadActFuncSet instead of ~300)."""
    orig = _bacc_mod.get_activation_tables

    def pinned(arch):
        tabs = orig(arch)
        keep = "natural_log_exp_and_others"
        if keep in tabs:
            tabs = {k: (v if k == keep else set()) for k, v in tabs.items()}
        return tabs

    _bacc_mod.get_activation_tables = pinned
    try:
        yield
    finally:
        _bacc_mod.get_activation_tables = orig


BF16 = ml_dtypes.bfloat16
FP8 = ml_dtypes.float8_e4m3    # TRN float8e4-compatible for |x| <= 240
dt = mybir.dt
P = 128
CH = 256                       # edge slots per DoubleRow chunk (2 k-tiles)
N_CORES = 8
SM = 64.0                      # edge-weight scale (fp8 denormal avoidance)
SV = 16.0                      # layer-2 table scale

import os as _os
GMAX = int(_os.environ.get("K_GMAX", "32"))  # k-tiles per dma_gather call
# multi-packet gathers: single_packet=True hangs the DGE above ~1024
# descriptors per call; multi-packet handles a full (tile, half) call.
SPKT = bool(_os.environ.get("K_SP"))
USE_DR = not _os.environ.get("K_NODR")        # DoubleRow fp8 matmuls


# ----------------------------------------------------------------------------
# Host-side edge preprocessing
# ----------------------------------------------------------------------------

def preprocess_edges(edge_src, edge_dst, edge_w, N, S):
    """Group edges per (dst 128-tile, src half); slots are 128-aligned per
    half, padded to the max core count so the program is uniform across
    cores. Within a tile, slots run lo-half k-tiles then hi-half k-tiles,
    then pad k-tiles to an even (DoubleRow) count.

    Returns (ktl, kth, M_list, idxl_list, idxh_list).
    """
    edge_src = np.asarray(edge_src).astype(np.int64)
    edge_dst = np.asarray(edge_dst).astype(np.int64)
    edge_w = np.asarray(edge_w, dtype=np.float32)
    n_tiles = (S + P - 1) // P
    HALF = N // 2

    per_core = []
    cnt = np.zeros((N_CORES, n_tiles, 2), dtype=np.int64)
    for c in range(N_CORES):
        sel = (edge_dst >= c * S) & (edge_dst < (c + 1) * S)
        dl = edge_dst[sel] - c * S
        sg = edge_src[sel]
        w = edge_w[sel]
        hi = (sg >= HALF).astype(np.int64)
        t = dl >> 7
        order = np.lexsort((dl, hi, t))
        dl, sg, w, hi, t = dl[order], sg[order], w[order], hi[order], t[order]
        np.add.at(cnt, (c, t[hi == 0], 0), 1)
        np.add.at(cnt, (c, t[hi == 1], 1), 1)
        per_core.append((dl, sg, w, hi, t))

    mx = cnt.max(axis=0)                                   # [n_tiles, 2]
    assert cnt.min() >= 1, "empty (tile, half) group: gather needs >=1 idx"
    ktl = ((mx[:, 0] + P - 1) // P).astype(np.int64)       # lo k-tiles/tile
    kth = ((mx[:, 1] + P - 1) // P).astype(np.int64)       # hi k-tiles/tile
    kth += (ktl + kth) & 1     # even k-tile count: every slot is gathered
    ch_t = (ktl + kth) // 2                                # DR chunks/tile
    cbase = np.concatenate([[0], np.cumsum(ch_t)])
    nchunk = int(cbase[-1])
    klo_base = np.concatenate([[0], np.cumsum(ktl)])
    khi_base = np.concatenate([[0], np.cumsum(kth)])
    KLO, KHI = int(klo_base[-1]), int(khi_base[-1])

    M_list, idxl_list, idxh_list = [], [], []
    for c in range(N_CORES):
        dl, sg, w, hi, t = per_core[c]
        # position within each (t, h) group
        gid = t * 2 + hi
        gcnt = np.bincount(gid, minlength=2 * n_tiles)
        gstart = np.concatenate([[0], np.cumsum(gcnt)])
        pos = np.arange(len(dl)) - gstart[gid]
        # slot within the tile: lo edges first, then hi
        slot = np.where(hi == 0, pos, ktl[t] * P + pos)
        kt = slot >> 7
        lane = slot & 127
        chunk = cbase[t] + (kt >> 1)
        M = np.zeros((P, nchunk * CH), dtype=FP8)
        M[lane, chunk * CH + (kt & 1) * P + (dl & 127)] = (w * SM).astype(FP8)

        idxl = np.zeros((P, KLO * 8), dtype=np.int16)
        idxh = np.zeros((P, max(KHI, 1) * 8), dtype=np.int16)
        for (arr, msk, off, kbase) in ((idxl, hi == 0, 0, klo_base),
                                       (idxh, hi == 1, HALF, khi_base)):
            # j: slot within this half's gather call for tile t
            j = pos[msk]
            col = kbase[t[msk]] * 8 + (j >> 4)
            row = j & 15
            val = (sg[msk] - off).astype(np.int16)
            for g in range(8):
                arr[16 * g + row, col] = val
        M_list.append(M)
        idxl_list.append(idxl)
        idxh_list.append(idxh)
    return (ktl, kth, M_list, idxl_list, idxh_list)


def pack_x(x0c, x1c, S):
    """[S, F0] x2 -> [128, n_tiles * 8 * 128] bf16: per tile, the 4+4
    feature-major [128, 128] lhsT chunks of both branches, contiguous so
    phase A needs a single DMA per tile."""
    F0 = x0c.shape[1]
    KF = F0 // P
    n_tiles = (S + P - 1) // P
    Sp = n_tiles * P
    buf = np.zeros((Sp, 2 * F0), dtype=BF16)
    buf[:S, :F0] = x0c
    buf[:S, F0:] = x1c
    # [t, c, j, p] -> [p, t, j, c]
    a = buf.reshape(n_tiles, P, 2 * KF, P).transpose(3, 0, 2, 1)
    return np.ascontiguousarray(a.reshape(P, n_tiles * 2 * KF * P))


# ----------------------------------------------------------------------------
# Bass program
# ----------------------------------------------------------------------------

def build_nc(N, F0, H, C, S, ktl, kth, single_core=False):
    n_tiles = (S + P - 1) // P
    HALF = N // 2
    ch_t = (ktl + kth + 1) // 2
    cbase = np.concatenate([[0], np.cumsum(ch_t)])
    nchunk = int(cbase[-1])
    klo_base = np.concatenate([[0], np.cumsum(ktl)])
    khi_base = np.concatenate([[0], np.cumsum(kth)])
    KLO, KHI = int(klo_base[-1]), int(khi_base[-1])
    KMAX = int((2 * ch_t).max())              # k-tiles per tile incl pad
    KF = F0 // P
    KH = H // P
    H2 = 2 * H
    XW = 2 * KF * P                           # packed-x columns per tile
    DR = mybir.MatmulPerfMode.DoubleRow

    nc = bacc.Bacc("TRN2", num_devices=1 if single_core else N_CORES,
                   dynamic_dma_scratch_size=98304)

    # --- I/O ---
    XP = nc.declare_dram_parameter("XP", [P, n_tiles * XW], dt.bfloat16, isOutput=False)
    W1a = nc.declare_dram_parameter("W1a", [F0, H], dt.bfloat16, isOutput=False)
    W1b = nc.declare_dram_parameter("W1b", [F0, H], dt.bfloat16, isOutput=False)
    W2a = nc.declare_dram_parameter("W2a", [H, H], dt.bfloat16, isOutput=False)
    W2b = nc.declare_dram_parameter("W2b", [H, H], dt.bfloat16, isOutput=False)
    LWa = nc.declare_dram_parameter("LWa", [H, H], dt.bfloat16, isOutput=False)
    LWb = nc.declare_dram_parameter("LWb", [H, H], dt.bfloat16, isOutput=False)
    LWf = nc.declare_dram_parameter("LWf", [H2, C], dt.bfloat16, isOutput=False)
    b1 = nc.declare_dram_parameter("b1", [P, H2], dt.bfloat16, isOutput=False)
    b2 = nc.declare_dram_parameter("b2", [P, H2], dt.bfloat16, isOutput=False)
    lba = nc.declare_dram_parameter("lba", [P, H], dt.bfloat16, isOutput=False)
    lbb = nc.declare_dram_parameter("lbb", [P, H], dt.bfloat16, isOutput=False)
    lbf = nc.declare_dram_parameter("lbf", [P, C], dt.bfloat16, isOutput=False)
    Mt = nc.declare_dram_parameter("M", [P, nchunk * CH], dt.float8e4, isOutput=False)
    IDXL = nc.declare_dram_parameter("IDXL", [P, KLO * 8], dt.int16, isOutput=False)
    IDXH = nc.declare_dram_parameter("IDXH", [P, max(KHI, 1) * 8], dt.int16, isOutput=False)
    out_t = nc.declare_dram_parameter("out", [S, C], dt.float32, isOutput=True)

    # --- internal DRAM ---
    u_loc = nc.dram_tensor("u_loc", [S, H2], dt.float8e4)
    v_loc = nc.dram_tensor("v_loc", [S, H2], dt.float8e4)
    if single_core:
        U = nc.declare_dram_parameter("Uin", [N, H2], dt.float8e4, isOutput=False)
        V = nc.declare_dram_parameter("Vin", [N, H2], dt.float8e4, isOutput=False)
    else:
        U = nc.dram_tensor("U", [N, H2], dt.float8e4, addr_space="Shared")
        V = nc.dram_tensor("V", [N, H2], dt.float8e4, addr_space="Shared")
    groups = [list(range(N_CORES))]

    with TileContext(nc, num_cores=N_CORES) as tc:
        ctx = contextlib.ExitStack()
        with ctx:
            perm = ctx.enter_context(tc.tile_pool(name="perm", bufs=1))
            xp = ctx.enter_context(tc.tile_pool(name="xp", bufs=3))
            hp = ctx.enter_context(tc.tile_pool(name="hp", bufs=3))
            mpool = ctx.enter_context(tc.tile_pool(name="mpool", bufs=3))
            msgp = ctx.enter_context(tc.tile_pool(name="msgp", bufs=3))
            sb = ctx.enter_context(tc.tile_pool(name="sb", bufs=2))
            stat = ctx.enter_context(tc.tile_pool(name="stat", bufs=4))
            ps_big = ctx.enter_context(tc.tile_pool(name="ps_big", bufs=2, space="PSUM"))
            ps_d = ctx.enter_context(tc.tile_pool(name="ps_d", bufs=2, space="PSUM"))
            ps_t = ctx.enter_context(tc.tile_pool(name="ps_t", bufs=2, space="PSUM"))
            ps_f = ctx.enter_context(tc.tile_pool(name="ps_f", bufs=2, space="PSUM"))

            # persistent small tiles
            ident = perm.tile([P, P], dt.bfloat16, tag="ident")
            make_identity(nc, ident[:])
            w1a_t = [perm.tile([P, H], dt.bfloat16, name=f"w1a{k}", tag=f"w1a{k}") for k in range(KF)]
            w1b_t = [perm.tile([P, H], dt.bfloat16, name=f"w1b{k}", tag=f"w1b{k}") for k in range(KF)]
            w2a_t = [perm.tile([P, H], dt.bfloat16, name=f"w2a{k}", tag=f"w2a{k}") for k in range(KH)]
            w2b_t = [perm.tile([P, H], dt.bfloat16, name=f"w2b{k}", tag=f"w2b{k}") for k in range(KH)]
            lwa_t = [perm.tile([P, H], dt.bfloat16, name=f"lwa{k}", tag=f"lwa{k}") for k in range(KH)]
            lwb_t = [perm.tile([P, H], dt.bfloat16, name=f"lwb{k}", tag=f"lwb{k}") for k in range(KH)]
            lwf_t = [perm.tile([P, C], dt.bfloat16, name=f"lwf{k}", tag=f"lwf{k}") for k in range(2 * KH)]
            for k in range(KF):
                nc.sync.dma_start(out=w1a_t[k][:], in_=W1a[k * P:(k + 1) * P, :])
                nc.sync.dma_start(out=w1b_t[k][:], in_=W1b[k * P:(k + 1) * P, :])
            for k in range(KH):
                nc.sync.dma_start(out=w2a_t[k][:], in_=W2a[k * P:(k + 1) * P, :])
                nc.sync.dma_start(out=w2b_t[k][:], in_=W2b[k * P:(k + 1) * P, :])
                nc.sync.dma_start(out=lwa_t[k][:], in_=LWa[k * P:(k + 1) * P, :])
                nc.sync.dma_start(out=lwb_t[k][:], in_=LWb[k * P:(k + 1) * P, :])
            for k in range(2 * KH):
                nc.sync.dma_start(out=lwf_t[k][:], in_=LWf[k * P:(k + 1) * P, :])
            b1_t = perm.tile([P, H2], dt.bfloat16, tag="b1")
            b2_t = perm.tile([P, H2], dt.bfloat16, tag="b2")
            lba_t = perm.tile([P, H], dt.bfloat16, tag="lba")
            lbb_t = perm.tile([P, H], dt.bfloat16, tag="lbb")
            lbf_t = perm.tile([P, C], dt.bfloat16, tag="lbf")
            nc.sync.dma_start(out=b1_t[:], in_=b1[:])
            nc.sync.dma_start(out=b2_t[:], in_=b2[:])
            nc.sync.dma_start(out=lba_t[:], in_=lba[:])
            nc.sync.dma_start(out=lbb_t[:], in_=lbb[:])
            nc.sync.dma_start(out=lbf_t[:], in_=lbf[:])
            idxl_t = perm.tile([P, KLO * 8], dt.int16, tag="idxl")
            nc.sync.dma_start(out=idxl_t[:], in_=IDXL[:])
            idxh_t = perm.tile([P, max(KHI, 1) * 8], dt.int16, tag="idxh")
            nc.sync.dma_start(out=idxh_t[:], in_=IDXH[:])

            def mtile(m):
                ms = m * P
                return ms, min(P, S - ms)

            # ---------------- Phase A: u = x @ W1 (both branches) ----------
            for m in range(n_tiles):
                ms, mw = mtile(m)
                xa = xp.tile([P, XW], dt.bfloat16, tag="xa")
                nc.sync.dma_start(out=xa[:], in_=XP[:, m * XW:(m + 1) * XW])
                pa = ps_d.tile([P, H], dt.float32, tag="ps_d")
                pb = ps_d.tile([P, H], dt.float32, tag="ps_d")
                for k in range(KF):
                    nc.tensor.matmul(pa[:mw, :], lhsT=xa[:, k * P:k * P + mw],
                                     rhs=w1a_t[k][:], start=(k == 0), stop=(k == KF - 1))
                for k in range(KF):
                    nc.tensor.matmul(pb[:mw, :], lhsT=xa[:, (KF + k) * P:(KF + k) * P + mw],
                                     rhs=w1b_t[k][:], start=(k == 0), stop=(k == KF - 1))
                uab = sb.tile([P, H2], dt.float8e4, tag="uab")
                nc.scalar.activation(out=uab[:mw, :H], in_=pa[:mw, :],
                                     func=mybir.ActivationFunctionType.Copy)
                nc.scalar.activation(out=uab[:mw, H:], in_=pb[:mw, :],
                                     func=mybir.ActivationFunctionType.Copy)
                nc.sync.dma_start(out=u_loc[ms:ms + mw, :], in_=uab[:mw, :])

            # ---------------- AllGather u ---------------------------------
            if not single_core:
                nc.gpsimd.collective_compute(
                    "AllGather", mybir.AluOpType.bypass, replica_groups=groups,
                    ins=[u_loc[:]], outs=[U[:]])

            # ---------------- spmm tile emitter ---------------------------
            def emit_spmm_tile(t, table, bias_t, relu, outT, mtag):
                """outT: [P, 2*KH*P] tile receiving the feature-major result
                (a-branch chunks then b-branch chunks)."""
                ts_, tw = mtile(t)
                nkl, nkh = int(ktl[t]), int(kth[t])
                nch = int(ch_t[t])
                cb = int(cbase[t])
                mt = mpool.tile([P, nch * CH], dt.float8e4, tag="mt")
                nc.sync.dma_start(out=mt[:], in_=Mt[:, cb * CH:(cb + nch) * CH])
                msg = msgp.tile([P, KMAX * H2], dt.float8e4, tag="msg")
                for (base, kn, itile, ibase) in (
                        (0, nkl, idxl_t, klo_base[t]),
                        (nkl, nkh, idxh_t, khi_base[t])):
                    tbl = table[:HALF, :] if base == 0 else table[HALF:, :]
                    for a in range(0, kn, GMAX):
                        b = min(a + GMAX, kn)
                        nc.gpsimd.dma_gather(
                            out_ap=msg[:, (base + a) * H2:(base + b) * H2]
                                .rearrange("p (n e) -> p n e", e=H2),
                            in_ap=tbl,
                            idxs_ap=itile[:, (ibase + a) * 8:(ibase + b) * 8],
                            num_idxs=(b - a) * P, num_idxs_reg=(b - a) * P,
                            elem_size=H2, single_packet=SPKT)
                ph = ps_big.tile([P, H2], dt.float32, tag="ps_big")
                if USE_DR:
                    for j in range(nch):
                        nc.tensor.matmul(
                            ph[:, :],
                            lhsT=mt[:, j * CH:(j + 1) * CH].rearrange(
                                "p (two m) -> p two m", two=2),
                            rhs=msg[:, j * 2 * H2:(j + 1) * 2 * H2].rearrange(
                                "p (two e) -> p two e", two=2),
                            start=(j == 0), stop=(j == nch - 1), perf_mode=DR)
                else:
                    nk = 2 * nch
                    for j in range(nk):
                        nc.tensor.matmul(
                            ph[:, :],
                            lhsT=mt[:, j * P:(j + 1) * P],
                            rhs=msg[:, j * H2:(j + 1) * H2],
                            start=(j == 0), stop=(j == nk - 1))
                hab = sb.tile([P, H2], dt.bfloat16, tag=mtag)
                nc.vector.tensor_tensor(out=hab[:tw, :], in0=ph[:tw, :],
                                        in1=bias_t[:tw, :],
                                        op=mybir.AluOpType.add)
                if relu:
                    nc.vector.tensor_scalar_max(hab[:tw, :], hab[:tw, :], 0.0)
                for fc in range(2 * KH):
                    pt = ps_t.tile([P, P], dt.bfloat16, tag="ps_t")
                    nc.tensor.transpose(out=pt[:, :tw],
                                        in_=hab[:tw, fc * P:(fc + 1) * P],
                                        identity=ident[:tw, :tw])
                    nc.vector.tensor_scalar_add(
                        outT[:, fc * P:fc * P + tw], pt[:, :tw], 0.0)

            # ---------------- layer 1: h = relu(spmm(U) + b1'); v = h @ W2'
            for t in range(n_tiles):
                ms, mw = mtile(t)
                hT = hp.tile([P, 2 * KH * P], dt.bfloat16, tag="hT")
                emit_spmm_tile(t, U, b1_t, True, hT, "hab")
                # phase D for this tile (fills PE bubbles during gathers)
                pa = ps_d.tile([P, H], dt.float32, tag="ps_d")
                pb = ps_d.tile([P, H], dt.float32, tag="ps_d")
                for k in range(KH):
                    nc.tensor.matmul(pa[:mw, :], lhsT=hT[:, k * P:k * P + mw],
                                     rhs=w2a_t[k][:], start=(k == 0), stop=(k == KH - 1))
                for k in range(KH):
                    nc.tensor.matmul(pb[:mw, :], lhsT=hT[:, (KH + k) * P:(KH + k) * P + mw],
                                     rhs=w2b_t[k][:], start=(k == 0), stop=(k == KH - 1))
                vab = sb.tile([P, H2], dt.float8e4, tag="vab")
                nc.scalar.activation(out=vab[:mw, :H], in_=pa[:mw, :],
                                     func=mybir.ActivationFunctionType.Copy)
                nc.scalar.activation(out=vab[:mw, H:], in_=pb[:mw, :],
                                     func=mybir.ActivationFunctionType.Copy)
                nc.sync.dma_start(out=v_loc[ms:ms + mw, :], in_=vab[:mw, :])

            # ---------------- AllGather v ---------------------------------
            if not single_core:
                nc.gpsimd.collective_compute(
                    "AllGather", mybir.AluOpType.bypass, replica_groups=groups,
                    ins=[v_loc[:]], outs=[V[:]])

            # ---------------- layer 2 + heads -----------------------------
            def softmax_z(py, lb_t, zdst, mw, width):
                """zdst <- log_softmax(py + lb) ; py is PSUM [P, width] f32."""
                yf = sb.tile([P, width], dt.float32, tag=f"yf{width}")
                nc.vector.tensor_tensor(out=yf[:mw, :], in0=py[:mw, :],
                                        in1=lb_t[:mw, :], op=mybir.AluOpType.add)
                nmx = stat.tile([P, 1], dt.float32, tag="nmx")
                nc.vector.tensor_reduce(out=nmx[:mw, :], in_=yf[:mw, :],
                                        axis=mybir.AxisListType.X,
                                        op=mybir.AluOpType.max, negate=True)
                ex = sb.tile([P, width], dt.float32, tag=f"ex{width}")
                sx = stat.tile([P, 1], dt.float32, tag="sx")
                nc.scalar.activation(out=ex[:mw, :], in_=yf[:mw, :],
                                     func=mybir.ActivationFunctionType.Exp,
                                     bias=nmx[:mw, :], scale=1.0,
                                     accum_out=sx[:mw, :])
                lse = stat.tile([P, 1], dt.float32, tag="lse")
                nc.scalar.activation(out=lse[:mw, :], in_=sx[:mw, :],
                                     func=mybir.ActivationFunctionType.Ln)
                nc.vector.tensor_scalar(out=zdst, in0=yf[:mw, :],
                                        scalar1=nmx[:mw, :], scalar2=lse[:mw, :],
                                        op0=mybir.AluOpType.add,
                                        op1=mybir.AluOpType.subtract)

            for t in range(n_tiles):
                ms, mw = mtile(t)
                gT = hp.tile([P, 2 * KH * P], dt.bfloat16, tag="gT")
                emit_spmm_tile(t, V, b2_t, False, gT, "gab")
                # phase G: z = log_softmax(g @ LW' + Lb)
                zab = sb.tile([P, H2], dt.bfloat16, tag="zab")
                for br, lw_t, lb_t in ((0, lwa_t, lba_t), (1, lwb_t, lbb_t)):
                    py = ps_d.tile([P, H], dt.float32, tag="ps_d")
                    for k in range(KH):
                        nc.tensor.matmul(
                            py[:mw, :],
                            lhsT=gT[:, (br * KH + k) * P:(br * KH + k) * P + mw],
                            rhs=lw_t[k][:], start=(k == 0), stop=(k == KH - 1))
                    softmax_z(py, lb_t, zab[:mw, br * H:(br + 1) * H], mw, H)
                zTt = hp.tile([P, 2 * KH * P], dt.bfloat16, tag="zT")
                for fc in range(2 * KH):
                    pt = ps_t.tile([P, P], dt.bfloat16, tag="ps_t")
                    nc.tensor.transpose(out=pt[:, :mw],
                                        in_=zab[:mw, fc * P:(fc + 1) * P],
                                        identity=ident[:mw, :mw])
                    nc.vector.tensor_scalar_add(
                        zTt[:, fc * P:fc * P + mw], pt[:, :mw], 0.0)
                # phase H: out = log_softmax(z @ LWf + Lb)
                pf = ps_f.tile([P, C], dt.float32, tag="ps_f")
                for k in range(2 * KH):
                    nc.tensor.matmul(pf[:mw, :], lhsT=zTt[:, k * P:k * P + mw],
                                     rhs=lwf_t[k][:], start=(k == 0),
                                     stop=(k == 2 * KH - 1))
                ot = sb.tile([P, C], dt.float32, tag="ot")
                softmax_z(pf, lbf_t, ot[:mw, :], mw, C)
                nc.sync.dma_start(out=out_t[ms:ms + mw, :], in_=ot[:mw, :])

    import os
    if os.environ.get("NO_ACT_PIN"):
        nc.compile()
    else:
        with _pinned_act_tables():
            nc.compile()
    return nc


# ----------------------------------------------------------------------------
# Entry point
# ----------------------------------------------------------------------------

_CACHE = {}


def kernel(x0, x1, edge_src, edge_dst, edge_w,
           W1a, b1a, W2a, b2a, LWa, Lba,
           W1b, b1b, W2b, b2b, LWb, Lbb,
           LW, Lb):
    x0 = np.asarray(x0)
    x1 = np.asarray(x1)
    N, F0 = x0.shape
    H = np.asarray(W1a).shape[1]
    C = np.asarray(LW).shape[1]
    S = N // N_CORES

    key = (N, F0, H, C,
           hash(np.asarray(edge_src).tobytes()) ^ hash(np.asarray(edge_dst).tobytes()))
    if key not in _CACHE:
        ktl, kth, M_list, idxl_list, idxh_list = preprocess_edges(
            edge_src, edge_dst, edge_w, N, S)
        nc = build_nc(N, F0, H, C, S, ktl, kth)
        _CACHE[key] = (nc, M_list, idxl_list, idxh_list)
    nc, M_list, idxl_list, idxh_list = _CACHE[key]

    bf = lambda a: np.asarray(a, dtype=BF16)
    f32 = lambda a: np.asarray(a, dtype=np.float32)
    bcast = lambda v: np.broadcast_to(np.asarray(v, dtype=BF16)[None, :], (P, len(v))).copy()

    x0b = bf(x0)
    x1b = bf(x1)
    # fp8 spmm scale folding: M carries SM*w; layer-1 PSUM = SM*spmm(u), so
    # b1' = SM*b1 and h' = SM*h. W2' = W2*SV/SM makes v' = SV*v the fp8
    # layer-2 table; layer-2 PSUM = SM*SV*spmm(v), so b2' = SM*SV*b2 and
    # LW' = LW/(SM*SV) restores the exact reference logits.
    shared = {
        "W1a": bf(W1a), "W1b": bf(W1b),
        "W2a": bf(f32(W2a) * (SV / SM)), "W2b": bf(f32(W2b) * (SV / SM)),
        "LWa": bf(f32(LWa) / (SM * SV)), "LWb": bf(f32(LWb) / (SM * SV)),
        "LWf": bf(LW),
        "b1": bcast(np.concatenate([f32(b1a), f32(b1b)]) * SM),
        "b2": bcast(np.concatenate([f32(b2a), f32(b2b)]) * (SM * SV)),
        "lba": bcast(f32(Lba)), "lbb": bcast(f32(Lbb)), "lbf": bcast(f32(Lb)),
    }
    in_maps = []
    for c in range(N_CORES):
        in_maps.append({
            **shared,
            "XP": pack_x(x0b[c * S:(c + 1) * S], x1b[c * S:(c + 1) * S], S),
            "M": M_list[c], "IDXL": idxl_list[c], "IDXH": idxh_list[c],
        })
    res = run_bass_kernel_spmd(nc, in_maps, list(range(N_CORES)))
    return np.concatenate([res.results[c]["out"] for c in range(N_CORES)], axis=0)
